# revision 4
# baseline (speedup 1.0000x reference)
"""Trainium2 Bass kernel for nn_Connector_77738908057780 (dense_mlp).

Computation (see reference):
  x   = image_features                      [B, N, H]    bf16
  f1  = mean(hidden[0:13],  axis=0)         [B, N, H]
  f2  = mean(hidden[13:26], axis=0)         [B, N, H]
  cat = concat([x, f1, f2], -1)             [B, N, 3H]
  h   = gelu(cat @ W1.T + b1)               W1 = nf4_dequant(codes1, scales1) [H, 3H]
  fg  = h @ W2.T + b2                       W2 = nf4_dequant(codes2, scales2) [H, H]
  out = w * LN(fg) + (1-w) * LN(x),         w = sigmoid(alpha)

Sharding: data-parallel over batch B=8 -> one batch element per NeuronCore.

Per-core plan (v3). The v1 baseline (302us) was tensor-engine-bound: PE
sustains only ~1.2 GHz with a 50% util-limit throttle, and bf16 runs
1 col/cycle. This version halves the dominant GEMM1 s-channel work with
fp8 DoubleRow matmuls (2 k-tiles per instruction, ~0.565 cyc/col):
  - hidden loads and 13-layer DVE sums are token-major (HWDGE line rate,
    proven path); sums + x are xbar-transposed SBUF->SBUF into
    token-contiguous feature-major tiles [128, k, 256].
  - the s1/s2 halves land in one bf16 tile which DVE casts to fp8_e4m3;
    the x third stays bf16 (numerically validated: x-part bf16 + s-part
    fp8 + GEMM2 bf16 gives ~1.2e-2 rel err vs the 2e-2 gate; all-fp8
    fails at 4e-2; the mean's 1/13 is folded into the fp8 weights).
  - GEMM1 per output tile: 9 bf16 matmuls (x) + 9 fp8-DoubleRow matmuls
    (18 s k-tiles) accumulate one PSUM tile; exact-GELU(+b1) on ACT
    evacuates to g^T, which feeds GEMM2 (bf16, weights-moving) producing
    fg token-major. LayerNorm stats via bn_stats/bn_aggr, batched rsqrt,
    4 fused scalar_tensor_tensor combines. Stores ride the SWDGE ring.
  - supertiles of 256 tokens; the last overlaps by 39 (729 = 3*256-39)
    so every DMA/transpose tile is full-size.

NF4 dequant of the (small, replicated) weights is host-side weight prep.
"""

import os
import sys

import numpy as np
import ml_dtypes

for _p in ("/opt/trn_rl_repo", "/root/.axon_site/_ro/trn_rl_repo"):
    if os.path.isdir(_p) and _p not in sys.path:
        sys.path.insert(0, _p)

import concourse.bass as bass
import concourse.mybir as mybir
import concourse.tile as tile
from concourse import bacc
from concourse import bass_utils

BF16 = mybir.dt.bfloat16
FP8 = mybir.dt.float8e4
F32 = mybir.dt.float32
AF = mybir.ActivationFunctionType
ALU = mybir.AluOpType
DR = mybir.MatmulPerfMode.DoubleRow

NP_BF16 = ml_dtypes.bfloat16
NP_FP8 = ml_dtypes.float8_e4m3

P = 128
H = 1152
H3 = 3456
NT = 729          # tokens per core (N); B=8 cores
L = 26
MO = H // P       # 9 feature tiles per H
EPS = 1e-5
CH = H // 3       # 384: fg free-dim chunks
TSUP = 256        # tokens per supertile
NSUB = 2          # 128-token subtiles per supertile

# Supertiles of exactly 256 tokens; the last overlaps the previous by 39
# tokens (473..511 computed twice, identical values stored twice) so that
# every DMA/compute tile is a full-size tile.
SUPERTILES = [0, 256, 473]

NF4_CODEBOOK = np.array([
    -1.0, -0.6961928009986877, -0.5250730514526367, -0.39491748809814453,
    -0.28444138169288635, -0.18477343022823334, -0.09105003625154495, 0.0,
    0.07958029955625534, 0.16093020141124725, 0.24611230194568634,
    0.33791524171829224, 0.4407098591327667, 0.5626170039176941,
    0.7229568362236023, 1.0], dtype=np.float32)

BLOCK = 64


def _dequant_nf4(codes, scales):
    """Match reference: codebook lookup * per-64-block absmax, cast bf16."""
    out_f, in_f = codes.shape
    w = NF4_CODEBOOK[codes].reshape(out_f, in_f // BLOCK, BLOCK)
    w = w * scales[:, :, None].astype(np.float32)
    return w.reshape(out_f, in_f)  # float32 (caller casts)


def _build_program(act=AF.Gelu):
    nc = bacc.Bacc(
        "TRN2",
        target_bir_lowering=False,
        debug=False,
        num_devices=1,
    )
    x_d = nc.dram_tensor("x", (NT, H), BF16, kind="ExternalInput").ap()
    hid_d = nc.dram_tensor("hid", (L, NT, H), BF16, kind="ExternalInput").ap()
    w1x_d = nc.dram_tensor("w1x", (H, H), BF16, kind="ExternalInput").ap()
    w1s_d = nc.dram_tensor("w1s", (2 * H, H), FP8, kind="ExternalInput").ap()
    w2t_d = nc.dram_tensor("w2t", (H, H), BF16, kind="ExternalInput").ap()
    b1s_d = nc.dram_tensor("b1s", (P, MO), F32, kind="ExternalInput").ap()
    b2b_d = nc.dram_tensor("b2b", (P, H), F32, kind="ExternalInput").ap()
    g1b_d = nc.dram_tensor("g1b", (P, H), BF16, kind="ExternalInput").ap()
    g2b_d = nc.dram_tensor("g2b", (P, H), BF16, kind="ExternalInput").ap()
    bcb_d = nc.dram_tensor("bcb", (P, H), BF16, kind="ExternalInput").ap()
    out_d = nc.dram_tensor("out", (NT, H), BF16, kind="ExternalOutput").ap()

    with tile.TileContext(nc) as tc:
        _program(nc, tc, x_d, hid_d, w1x_d, w1s_d, w2t_d, b1s_d, b2b_d,
                 g1b_d, g2b_d, bcb_d, out_d, act)

    nc.compile()
    return nc


def _program(nc, tc, x_d, hid_d, w1x_d, w1s_d, w2t_d, b1s_d, b2b_d, g1b_d,
             g2b_d, bcb_d, out_d, act=AF.Gelu):
    with (
        tc.tile_pool(name="consts", bufs=1) as cpool,
        tc.tile_pool(name="hl", bufs=8) as hpool,
        tc.tile_pool(name="acc", bufs=2) as apool,
        tc.tile_pool(name="st", bufs=2) as stpool,
        tc.tile_pool(name="sc", bufs=2) as scpool,
        tc.tile_pool(name="xt", bufs=2) as xtpool,
        tc.tile_pool(name="gt", bufs=2) as gpool,
        tc.tile_pool(name="xn", bufs=2) as xnpool,
        tc.tile_pool(name="fg", bufs=3) as fgpool,
        tc.tile_pool(name="outp", bufs=3) as opool,
        tc.tile_pool(name="stats", bufs=2) as spool,
        tc.tile_pool(name="tmp", bufs=2) as tpool,
        tc.tile_pool(name="ps1", bufs=3, space="PSUM") as ps1pool,
        tc.tile_pool(name="ps2", bufs=3, space="PSUM") as ps2pool,
    ):
        # ---- constants (loaded once) ----
        w1x_sb = cpool.tile([P, MO, H], BF16)
        nc.scalar.dma_start(w1x_sb, w1x_d.rearrange("(a k) n -> k a n", k=P))
        w1s_sb = cpool.tile([P, MO, 2, H], FP8)
        nc.scalar.dma_start(
            w1s_sb, w1s_d.rearrange("(a two k) n -> k a two n", k=P, two=2))
        w2t_sb = cpool.tile([P, MO, H], BF16)
        nc.sync.dma_start(w2t_sb, w2t_d.rearrange("(a k) n -> k a n", k=P))
        b1s_sb = cpool.tile([P, MO], F32)
        nc.scalar.dma_start(b1s_sb, b1s_d)
        b2b_sb = cpool.tile([P, H], F32)
        nc.scalar.dma_start(b2b_sb, b2b_d)
        g1b_sb = cpool.tile([P, H], BF16)
        nc.scalar.dma_start(g1b_sb, g1b_d)
        g2b_sb = cpool.tile([P, H], BF16)
        nc.scalar.dma_start(g2b_sb, g2b_d)
        bcb_sb = cpool.tile([P, H], BF16)
        nc.scalar.dma_start(bcb_sb, bcb_d)

        for st_idx, t0 in enumerate(SUPERTILES):
            # ---- x (token-major, also used by LN1) ----
            x_nat = xnpool.tile([P, NSUB, H], BF16, tag="xnat")
            nc.scalar.dma_start(
                x_nat,
                x_d[t0:t0 + TSUP, :].rearrange("(s p) f -> p s f", p=P),
            )

            # x^T: token-contiguous feature-major [128f, 9k, 256t]
            xT = xtpool.tile([P, MO, TSUP], BF16, tag="xT")
            for tt in range(NSUB):
                nc.scalar.dma_start_transpose(
                    xT[:, :, tt * P:(tt + 1) * P], x_nat[:, tt, :])

            # ---- 26-layer sums: plain HWDGE loads, DVE adds ----
            def accum_half(l_start, tag):
                layers = []
                for i in range(13):
                    lt = hpool.tile([P, NSUB, H], BF16, name=f"hl{tag}{i}",
                                    tag="hl")
                    eng = nc.sync if (i % 2 == 0) else nc.scalar
                    eng.dma_start(
                        lt,
                        hid_d[l_start + i, t0:t0 + TSUP, :].rearrange(
                            "(s p) f -> p s f", p=P))
                    layers.append(lt)
                acc = apool.tile([P, NSUB, H], BF16, name=f"s{tag}", tag=tag)
                nc.vector.tensor_add(acc, layers[0], layers[1])
                for i in range(2, 13):
                    nc.vector.tensor_add(acc, acc, layers[i])
                return acc

            s1 = accum_half(0, "s1")
            s2 = accum_half(13, "s2")

            # ---- s^T via xbar transpose, then DVE cast to fp8 ----
            sT = stpool.tile([P, 2 * MO, TSUP], BF16, tag="sT")
            for tt in range(NSUB):
                nc.sync.dma_start_transpose(
                    sT[:, 0:MO, tt * P:(tt + 1) * P], s1[:, tt, :])
                nc.sync.dma_start_transpose(
                    sT[:, MO:2 * MO, tt * P:(tt + 1) * P], s2[:, tt, :])
            sc8 = scpool.tile([P, 2 * MO, TSUP], FP8, tag="sc")
            nc.vector.tensor_scalar_mul(sc8, sT, 1.0)

            # ---- GEMM1: h^T; x-part bf16, s-part fp8 DoubleRow ----
            gT = gpool.tile([P, MO, TSUP], BF16, tag="gT")
            for oh in range(MO):
                ps1 = ps1pool.tile([P, TSUP], F32, tag="ps1")
                osl = slice(oh * P, (oh + 1) * P)
                for a in range(MO):
                    nc.tensor.matmul(
                        ps1,
                        lhsT=w1x_sb[:, a, osl],
                        rhs=xT[:, a, :],
                        start=(a == 0), stop=False,
                    )
                for a in range(MO):
                    nc.tensor.matmul(
                        ps1,
                        lhsT=w1s_sb[:, a, :, osl],
                        rhs=sc8[:, 2 * a:2 * a + 2, :],
                        perf_mode=DR,
                        start=False, stop=(a == MO - 1),
                    )
                nc.scalar.activation(gT[:, oh, :], ps1, act,
                                     bias=b1s_sb[:, oh:oh + 1])

            # ---- GEMM2 (+b2) per subtile; LN stats ----
            agg = spool.tile([P, NSUB, 4], F32, tag="agg")
            rpack = spool.tile([P, 2 * NSUB], F32, tag="rpack")
            fgs = []
            for tt in range(NSUB):
                fg = fgpool.tile([P, H], BF16, tag="fg")
                fgs.append(fg)
                for nn in range(3):
                    ps2 = ps2pool.tile([P, CH], F32, tag="ps2")
                    for kh in range(MO):
                        nc.tensor.matmul(
                            ps2,
                            lhsT=gT[:, kh, tt * P:(tt + 1) * P],
                            rhs=w2t_sb[:, kh, nn * CH:(nn + 1) * CH],
                            start=(kh == 0), stop=(kh == MO - 1),
                        )
                    nc.vector.tensor_tensor(
                        fg[:, nn * CH:(nn + 1) * CH], ps2,
                        b2b_sb[:, nn * CH:(nn + 1) * CH], ALU.add)

                bnf = spool.tile([P, 3, 6], F32, tag="bnf")
                for c3 in range(3):
                    nc.vector.bn_stats(bnf[:, c3, :],
                                       fg[:, c3 * CH:(c3 + 1) * CH])
                nc.vector.bn_aggr(agg[:, tt, 2:4], bnf)
                bnx = spool.tile([P, 3, 6], F32, tag="bnx")
                for c3 in range(3):
                    nc.vector.bn_stats(bnx[:, c3, :],
                                       x_nat[:, tt, c3 * CH:(c3 + 1) * CH])
                nc.vector.bn_aggr(agg[:, tt, 0:2], bnx)
                nc.vector.tensor_scalar_add(rpack[:, 2 * tt:2 * tt + 1],
                                            agg[:, tt, 1:2], EPS)
                nc.vector.tensor_scalar_add(rpack[:, 2 * tt + 1:2 * tt + 2],
                                            agg[:, tt, 3:4], EPS)

            # ---- rsqrt batched: one reciprocal (DVE) + one sqrt (ACT) ----
            ig = spool.tile([P, 2 * NSUB], F32, tag="ig")
            nc.vector.reciprocal(ig, rpack)
            nc.scalar.activation(ig, ig, AF.Sqrt)

            # ---- normalize + sigmoid gate, store (SWDGE ring) ----
            for tt in range(NSUB):
                fg = fgs[tt]
                tmp1 = tpool.tile([P, H], BF16, tag="tmp1")
                tmp2 = tpool.tile([P, H], BF16, tag="tmp2")
                # tmp1 = (x - mu1) * G1;  G1 = (1-w)*ln1_g  (broadcast)
                nc.vector.scalar_tensor_tensor(
                    tmp1, x_nat[:, tt, :], agg[:, tt, 0:1], g1b_sb,
                    ALU.subtract, ALU.mult)
                # tmp2 = (fg - mu2) * G2;  G2 = w*ln2_g
                nc.vector.scalar_tensor_tensor(
                    tmp2, fg, agg[:, tt, 2:3], g2b_sb,
                    ALU.subtract, ALU.mult)
                # tmp1 = tmp1 * ig1 + Bc;  Bc = w*ln2_b + (1-w)*ln1_b
                nc.vector.scalar_tensor_tensor(
                    tmp1, tmp1, ig[:, 2 * tt:2 * tt + 1], bcb_sb,
                    ALU.mult, ALU.add)
                # out = tmp2 * ig2 + tmp1
                out_t = opool.tile([P, H], BF16, tag="outt")
                nc.vector.scalar_tensor_tensor(
                    out_t, tmp2, ig[:, 2 * tt + 1:2 * tt + 2], tmp1,
                    ALU.mult, ALU.add)
                nc.gpsimd.dma_start(
                    out_d[t0 + tt * P:t0 + (tt + 1) * P, :], out_t)


_NC_CACHE = {}


def _get_nc():
    if "nc" not in _NC_CACHE:
        _NC_CACHE["nc"] = _build_program()
    return _NC_CACHE["nc"]


def _host_prep(codes1, scales1, b1, codes2, scales2, b2,
               ln1_g, ln1_b, ln2_g, ln2_b, alpha):
    # W1 columns: [x | f1 | f2]; the mean's 1/13 is folded into the s-blocks
    w1 = _dequant_nf4(codes1, scales1)
    # match reference rounding: dequant result is cast to bf16 first
    w1 = w1.astype(NP_BF16).astype(np.float32)
    w1x = np.ascontiguousarray(w1[:, :H].T).astype(NP_BF16)       # [H, H]
    w1s = np.ascontiguousarray(
        (w1[:, H:] * np.float32(1.0 / 13.0)).T).astype(NP_FP8)    # [2H, H]

    w2 = _dequant_nf4(codes2, scales2).astype(NP_BF16)
    w2t = np.ascontiguousarray(w2.astype(np.float32).T).astype(NP_BF16)

    b1s = np.ascontiguousarray(
        b1.astype(np.float32).reshape(MO, P).T)  # [P, MO]

    b2b = np.ascontiguousarray(
        np.broadcast_to(b2.astype(np.float32), (P, H)))

    a32 = alpha.astype(np.float32)
    w_gate = (1.0 / (1.0 + np.exp(-a32[0]))).astype(NP_BF16)
    one_minus = (NP_BF16(1.0) - w_gate)
    g1 = (one_minus.astype(np.float32) * ln1_g.astype(np.float32))
    g2 = (w_gate.astype(np.float32) * ln2_g.astype(np.float32))
    bc = (w_gate.astype(np.float32) * ln2_b.astype(np.float32)
          + one_minus.astype(np.float32) * ln1_b.astype(np.float32))
    g1b = np.ascontiguousarray(np.broadcast_to(g1.astype(NP_BF16), (P, H)))
    g2b = np.ascontiguousarray(np.broadcast_to(g2.astype(NP_BF16), (P, H)))
    bcb = np.ascontiguousarray(np.broadcast_to(bc.astype(NP_BF16), (P, H)))
    return w1x, w1s, w2t, b1s, b2b, g1b, g2b, bcb


def make_in_maps(image_features, hidden, codes1, scales1, b1, codes2, scales2,
                 b2, ln1_g, ln1_b, ln2_g, ln2_b, alpha):
    w1x, w1s, w2t, b1s, b2b, g1b, g2b, bcb = _host_prep(
        codes1, scales1, b1, codes2, scales2, b2,
        ln1_g, ln1_b, ln2_g, ln2_b, alpha)
    B = image_features.shape[0]
    in_maps = []
    for c in range(B):
        in_maps.append({
            "x": np.ascontiguousarray(image_features[c]).astype(NP_BF16, copy=False),
            "hid": np.ascontiguousarray(hidden[:, c]).astype(NP_BF16, copy=False),
            "w1x": w1x, "w1s": w1s, "w2t": w2t, "b1s": b1s, "b2b": b2b,
            "g1b": g1b, "g2b": g2b, "bcb": bcb,
        })
    return in_maps


def kernel(image_features, hidden, codes1, scales1, b1, codes2, scales2, b2,
           ln1_g, ln1_b, ln2_g, ln2_b, alpha, _trace=False):
    B, N, Hin = image_features.shape
    assert (B, N, Hin) == (8, NT, H), (B, N, Hin)
    nc = _get_nc()
    in_maps = make_in_maps(image_features, hidden, codes1, scales1, b1,
                           codes2, scales2, b2, ln1_g, ln1_b, ln2_g, ln2_b,
                           alpha)
    res = bass_utils.run_bass_kernel_spmd(
        nc, in_maps, core_ids=list(range(8)), trace=_trace)
    out = np.stack([res.results[c]["out"] for c in range(8)])
    if _trace:
        kernel._last_results = res
    return out.astype(image_features.dtype, copy=False)
